# revision 15
# baseline (speedup 1.0000x reference)
"""Trainium2 Bass kernel for nn_AttentionLayer_83545703842160.

Single-head attention over spatial tokens, per batch element:
  t = x[b].reshape(C, H*W).T            # [N, C], N=4096, C=64
  q,k,v = t@W{q,k,v}.T + b{q,k,v}
  out   = softmax(q@k.T / sqrt(C)) @ v  # -> [C, N] -> [C, H, W]

Sharding: data-parallel over batch B=8 across the 8 NeuronCores (one
batch element per core). Each core holds the full (tiny) QKV weights.

Per-core kernel (all in f32, matmuls in float32r for 1 cyc/row):
  - xt  [65, 4096] SBUF: x[b] in [C, N] layout + a ones row (row 64)
    so biases fold into the contraction.
  - qT,kT [64, 4096] = Wq_ext/Wk_ext @ xt (PE), copied PSUM->SBUF.
  - v_sb [128, 32, 65] token-major v with a ones column (col 64), via
    per-m-tile matmuls lhsT=xt-slice, rhs=Wv_ext.
  - main loop over 8 query superblocks (S=512) x 32 key tiles (128):
      MM1: sT[m-tile 128, S] = kT-slice.T @ qT-slice  (PSUM)
      ACT: exp(0.125 * sT) PSUM->SBUF in [128, 1536] chunks (3 m-tiles
           per ACTIVATE to amortize the ~290-cycle per-instr bubble)
      MM2: acc[65, S] += v_ext[m].T @ pT  (PSUM accumulate; row 64
           accumulates the softmax denominator via the ones column)
      tail: recip(rowsum) -> gpsimd partition_broadcast -> DVE multiply
            -> DMA out y[:, block]
  PSUM budget: scores 2x3 banks (ping-pong) + acc/qkv pool 2x1 = 8.

The ScalarE (ACT) engine is the bottleneck: softmax must exp N^2 =
16.7M elements/core at 1 elem/lane/cycle @ 1.2 GHz (~109 us floor).
Everything else (PE ~115 us of f32r matmul, DVE ~55 us, DMA ~10 us)
overlaps underneath it.
"""

import numpy as np
from contextlib import ExitStack

import concourse.bacc as bacc
import concourse.bass as bass
import concourse.mybir as mybir
import concourse.tile as tile
from concourse.bass import MemorySpace
from concourse.bass_utils import run_bass_kernel_spmd

C = 64          # channels
N = 4096        # tokens (64*64 spatial)
B = 8           # batch == number of cores
S = 512         # query superblock
MT = 128        # keys per m-tile
NMT = N // MT   # 32 m-tiles
FP32 = mybir.dt.float32
F32R = mybir.dt.float32r
EXP = mybir.ActivationFunctionType.Exp
# m-tiles per ACTIVATE chunk: 3 tiles -> [128, 1536] = 3 PSUM banks.
# The short group leads: its cheap exp is covered by the previous (long)
# group's exp while the PE refills the first slot after a superblock switch.
GROUPS = [2] + [3] * 10
assert sum(GROUPS) == NMT


def _build_kernel(tc, ctx, x_d, wq_d, wk_d, wv_d, y_d, reps=1):
    if reps > 1:
        # timing harness: repeat the whole body in a HW loop so kernel time
        # dominates dispatch overhead in wallclock measurements
        with tc.For_i(0, reps, 1):
            _build_body(tc, ctx, x_d, wq_d, wk_d, wv_d, y_d)
    else:
        _build_body(tc, ctx, x_d, wq_d, wk_d, wv_d, y_d)


def _build_body(tc, ctx, x_d, wq_d, wk_d, wv_d, y_d):
    nc = tc.nc

    sb = ctx.enter_context(tc.tile_pool(name="sb", bufs=1))
    # pt carries a full superblock (11 groups) between stage-1 and stage-2,
    # plus slack for the qkv phase overrunning into nsb1
    pt_pool = ctx.enter_context(tc.tile_pool(name="pt", bufs=14))
    osb_pool = ctx.enter_context(tc.tile_pool(name="osb", bufs=2))
    nrm_pool = ctx.enter_context(tc.tile_pool(name="nrm", bufs=2))
    sc_psum = ctx.enter_context(
        tc.tile_pool(name="scp", bufs=2, space=MemorySpace.PSUM))
    ac_psum = ctx.enter_context(
        tc.tile_pool(name="acp", bufs=2, space=MemorySpace.PSUM))

    xt = sb.tile([C + 1, N], F32R)
    wq_sb = sb.tile([C + 1, C], F32R)
    wk_sb = sb.tile([C + 1, C], F32R)
    wv_sb = sb.tile([C + 1, C + 2], F32R)
    qt = sb.tile([C, N], F32R)
    kt = sb.tile([C, N], F32R)
    v_sb = sb.tile([MT, NMT, C + 2], F32R)

    nc.sync.dma_start(wq_sb[:], wq_d)
    nc.sync.dma_start(wk_sb[:], wk_d)
    nc.sync.dma_start(wv_sb[:], wv_d)
    # x in column chunks so projections can start on chunk 0 early
    for j in range(N // S):
        nc.sync.dma_start(xt[:, j * S:(j + 1) * S],
                          x_d[:, j * S:(j + 1) * S])

    # Projection producers, emitted piecemeal so they interleave with the
    # exp stream instead of forming a serial head phase.
    def emit_qk(w_sb, dst, j):
        p = ac_psum.tile([C, S], FP32, tag="ps1")
        nc.tensor.matmul(p[:], w_sb[:], xt[:, j * S:(j + 1) * S],
                         start=True, stop=True)
        nc.vector.tensor_copy(dst[:, j * S:(j + 1) * S], p[:])

    def emit_v(m):
        p = ac_psum.tile([MT, C + 2], FP32, tag="ps1")
        nc.tensor.matmul(p[:], xt[:, m * MT:(m + 1) * MT], wv_sb[:],
                         start=True, stop=True)
        nc.vector.tensor_copy(v_sb[:, m, :], p[:])

    # head: only what the very first scores group needs
    emit_qk(wq_sb, qt, 0)
    emit_qk(wk_sb, kt, 0)

    # Stage-2 (attn @ v) lags stage-1 (scores+exp) by one superblock: during
    # nsb s the PE runs only MM1s (plus leftover projections in nsb0), so the
    # exp stream never waits on qkv/v or MM2 contention; the deep pt pool
    # carries a full superblock of exp'd scores between the stages.
    def emit_stage2(s, pts):
        acc = ac_psum.tile([C + 2, S], FP32, tag="ps1")
        m = 0
        for gi, gs in enumerate(GROUPS):
            for j in range(gs):
                mm = m + j
                nc.tensor.matmul(
                    acc[:], v_sb[:, mm, :], pts[gi][:, j * S:(j + 1) * S],
                    start=(mm == 0), stop=(mm == NMT - 1))
            m += gs
        # normalize: y[:, block] = acc[0:64] / acc[64] (denominator row)
        rs = nrm_pool.tile([1, S], FP32, tag="rs")
        nc.vector.tensor_copy(rs[:], acc[C:C + 1, :])
        rr = nrm_pool.tile([1, S], FP32, tag="rr")
        nc.vector.reciprocal(rr[:], rs[:])
        bc = nrm_pool.tile([C, S], FP32, tag="bc")
        nc.gpsimd.partition_broadcast(bc[:], rr[:], channels=C)
        ob = osb_pool.tile([C, S], FP32, tag="ob")
        nc.vector.tensor_mul(ob[:], acc[0:C, :], bc[:])
        nc.sync.dma_start(y_d[:, s * S:(s + 1) * S], ob[:])

    prev = None
    for s in range(N // S):
        qs = qt[:, s * S:(s + 1) * S]
        if s < N // S - 1:
            emit_qk(wq_sb, qt, s + 1)  # q chunk for the NEXT superblock
        pts = []
        m = 0
        for gi, gs in enumerate(GROUPS):
            if s == 0:
                # trickle in the remaining k chunks and the v tiles; k chunk
                # gi+1 lands one group before any MM1 needs it, and v is
                # consumed one whole superblock later (stage-2 lag), so
                # these producers never stall the exp stream
                if gi <= 6:
                    emit_qk(wk_sb, kt, gi + 1)
                if gi >= 1:
                    for mv in range(3 * (gi - 1), min(3 * gi, NMT)):
                        emit_v(mv)
            sc = sc_psum.tile([MT, gs * S], FP32, tag="sc")
            for j in range(gs):
                nc.tensor.matmul(
                    sc[:, j * S:(j + 1) * S],
                    kt[:, (m + j) * MT:(m + j + 1) * MT], qs,
                    start=True, stop=True)
            pt = pt_pool.tile([MT, gs * S], F32R, tag="pt")
            nc.scalar.activation(pt[:], sc[:], EXP, scale=0.125)
            pts.append(pt)
            m += gs
        if s == 0:
            for mv in range(3 * (len(GROUPS) - 1), NMT):
                emit_v(mv)
        if prev is not None:
            emit_stage2(prev[0], prev[1])
        prev = (s, pts)
    emit_stage2(prev[0], prev[1])


_NC_CACHE = {}


def _get_nc(reps=1):
    if reps not in _NC_CACHE:
        nc = bacc.Bacc("TRN2", target_bir_lowering=False, debug=False,
                       enable_asserts=False)
        x_d = nc.dram_tensor("x", [C + 1, N], F32R, kind="ExternalInput").ap()
        wq_d = nc.dram_tensor("wq", [C + 1, C], F32R,
                              kind="ExternalInput").ap()
        wk_d = nc.dram_tensor("wk", [C + 1, C], F32R,
                              kind="ExternalInput").ap()
        wv_d = nc.dram_tensor("wv", [C + 1, C + 2], F32R,
                              kind="ExternalInput").ap()
        y_d = nc.dram_tensor("y", [C, N], FP32, kind="ExternalOutput").ap()
        with tile.TileContext(nc) as tc:
            with ExitStack() as ctx:
                _build_kernel(tc, ctx, x_d, wq_d, wk_d, wv_d, y_d, reps=reps)
        nc.compile()
        _NC_CACHE[reps] = nc
    return _NC_CACHE[reps]


def _host_weights(Wq, bq, Wk, bk, Wv, bv):
    wq = np.concatenate([Wq.T, bq[None, :]], axis=0).astype(np.float32)
    wk = np.concatenate([Wk.T, bk[None, :]], axis=0).astype(np.float32)
    wv = np.zeros((C + 1, C + 2), np.float32)
    wv[:C, :C] = Wv.T
    wv[C, :C] = bv
    wv[C, C] = 1.0
    return np.ascontiguousarray(wq), np.ascontiguousarray(wk), wv


def _host_x(x_b):
    return np.ascontiguousarray(
        np.concatenate([x_b.reshape(C, N), np.ones((1, N), np.float32)],
                       axis=0))


def _run(inputs, reps=1, **spmd_kwargs):
    x = np.ascontiguousarray(np.asarray(inputs["x"], np.float32))
    wq, wk, wv = _host_weights(
        np.asarray(inputs["Wq"], np.float32),
        np.asarray(inputs["bq"], np.float32),
        np.asarray(inputs["Wk"], np.float32),
        np.asarray(inputs["bk"], np.float32),
        np.asarray(inputs["Wv"], np.float32),
        np.asarray(inputs["bv"], np.float32))
    nc = _get_nc(reps)
    in_maps = [
        {"x": _host_x(x[b]), "wq": wq, "wk": wk, "wv": wv}
        for b in range(B)
    ]
    res = run_bass_kernel_spmd(nc, in_maps, core_ids=list(range(B)),
                               **spmd_kwargs)
    out = np.stack([res.results[b]["y"].reshape(C, 64, 64)
                    for b in range(B)], axis=0)
    return out, res


def kernel(**inputs):
    out, _ = _run(inputs)
    return out
